# revision 1
# baseline (speedup 1.0000x reference)
"""Trainium2 Bass kernel for nn_Classifier (capsule-style conv + routing).

Math (validated against the jax reference):
  W = conv_w[:,0,:]                                   # [16, 640]
  y[b,i,o]   = relu(sum_t x[b,i,t] W[t,o] + conv_b[o])          (conv as matmul, K=16)
  U[b,k,i,d] = y[b,i,k*64+d]
  Usum[b,k,d]= sum_i U[b,k,i,d]
  logits     = (U . Usum)/4            -> stable softmax over i  -> C
  Cb         = C + B_bias[k,i]
  S[b,k,:]   = sum_i Cb[b,k,i] U[b,k,i,:]
  out[b,k]   = n2/(n2+1) with n2 = |S|^2   (the reference's
               (n2/(n2+1))*(sqrt(n2)/(sqrt(n2)+1e-7)) equals this to ~1e-8
               rel for any non-degenerate n2, avoiding Ln/Exp ACT tables)

Sharding: data-parallel over batch, 8 batches per core, 8 cores (SPMD).

v2 layout/structure (vs v1 baseline at ~101us):
  - host pre-transposes x into xT [2,128,512] (rows 32j+t, bias-fold row
    32j+16=1.0) and pre-builds w4/bbias/ident/gmask/smask -> no on-device
    setup phase (was ~18us of gpsimd affine_selects + PE transposes).
  - all matmul operands bf16 -> 1-pass PE (fp32/f32r ran LOW_HIGH 2-pass).
  - 4 warmup matmuls at t=0 overlap the input DMA and warm the PE HAM
    clock gate (half the baseline ran at 1.2GHz).
  - PSUM evictions (the real throughput floor: 41k elems/lane) spread
    across DVE / Pool / ACT by a weighted picker matching engine rates.
  - single strided output DMA (was 8 serialized 40-byte DMAs, 6.4us).
"""

import numpy as np

import concourse.bass as bass
import concourse.mybir as mybir
import concourse.tile as tile
from concourse import bacc
from concourse.bass_utils import run_bass_kernel_spmd

F32 = mybir.dt.float32
BF16 = mybir.dt.bfloat16

B_FULL = 64
N = 512          # num timecaps (routing dim i/m)
DT = 16          # dim timecaps (conv contraction)
K = 10           # classes
D = 64           # dim classes
NO = K * D       # 640 conv output channels
NCORES = 8
BPC = B_FULL // NCORES   # 8 batches per core
N_WARM = 4               # fp32 N=512 warmup matmuls (~3.4us cold => HAM warm)

NP_BF16 = mybir.dt.np(BF16)


class _EvictPicker:
    """Weighted round-robin over (engine, rate) for PSUM->SBUF evictions."""

    def __init__(self, nc):
        # GPSIMD cannot access PSUM -> DVE + ACT only.
        # approx elem/ns rates: DVE ~1.31, ACT ~0.77
        self.slots = [(nc.vector, 1.31), (nc.scalar, 0.77)]
        self.debt = [0.0, 0.0]
        self.nc = nc

    def pick(self, nelem, need_accum=False):
        best, best_t = None, None
        for idx, (eng, rate) in enumerate(self.slots):
            t = self.debt[idx] + nelem / rate
            if best_t is None or t < best_t:
                best, best_t = idx, t
        self.debt[best] = best_t
        return self.slots[best][0]


def _relu_evict(nc, eng, out, in_, accum_out=None):
    AF = mybir.ActivationFunctionType
    OP = mybir.AluOpType
    if eng is nc.scalar:
        nc.scalar.activation(out=out, in_=in_, func=AF.Relu, accum_out=accum_out)
    elif accum_out is not None:
        eng.tensor_scalar(out=out, in0=in_, scalar1=0.0, scalar2=0.0,
                          op0=OP.max, op1=OP.add, accum_out=accum_out)
    else:
        eng.tensor_scalar(out=out, in0=in_, scalar1=0.0, scalar2=None,
                          op0=OP.max)


def _build_program():
    nc = bacc.Bacc("TRN2", target_bir_lowering=False)
    xt_in = nc.declare_dram_parameter("xt", [2, 128, N], BF16, isOutput=False)
    w_in = nc.declare_dram_parameter("w", [128, NO], BF16, isOutput=False)
    bb_in = nc.declare_dram_parameter("bb", [128, N], F32, isOutput=False)
    id_in = nc.declare_dram_parameter("id", [128, 128], F32, isOutput=False)
    gm_in = nc.declare_dram_parameter("gm", [128, 5 * 32], F32, isOutput=False)
    sm_in = nc.declare_dram_parameter("sm", [128, NO], F32, isOutput=False)
    out_d = nc.declare_dram_parameter("out", [BPC, K], F32, isOutput=True)

    AF = mybir.ActivationFunctionType
    OP = mybir.AluOpType

    with tile.TileContext(nc) as tc:
        with tc.tile_pool(name="const", bufs=1) as cpool:
            xT = [cpool.tile([128, N], BF16, name=f"xT{g}", tag=f"xT{g}")
                  for g in range(2)]
            w_s = cpool.tile([128, NO], BF16, name="w_s", tag="w_s")
            bb_s = cpool.tile([128, N], F32, name="bb_s", tag="bb_s")
            ident = cpool.tile([128, 128], F32, name="ident", tag="ident")
            gmask = cpool.tile([128, 5 * 32], F32, name="gmask", tag="gmask")
            smask = cpool.tile([128, NO], F32, name="smask", tag="smask")

            # DMA order = priority order (single queue): warmup operand
            # first, then conv operands, then late-phase constants.
            nc.sync.dma_start(bb_s[:], bb_in[:, :])
            nc.sync.dma_start(w_s[:], w_in[:, :])
            for g in range(2):
                nc.sync.dma_start(xT[g][:], xt_in[g])
            nc.sync.dma_start(gmask[:], gm_in[:, :])
            nc.sync.dma_start(ident[:], id_in[:, :])
            nc.sync.dma_start(smask[:], sm_in[:, :])

            # ---- HAM warmup: fp32 2-pass N=512 matmuls ~853ns cold each ----
            with tc.tile_pool(name="ps_warm", bufs=1, space="PSUM") as pw:
                ps_w = pw.tile([128, N], F32, name="ps_w", tag="ps_w")
                for r in range(N_WARM):
                    nc.tensor.matmul(
                        ps_w[:], bb_s[:, 0:128], bb_s[:],
                        start=(r == 0), stop=(r == N_WARM - 1),
                    )

            with tc.tile_pool(name="data", bufs=1) as dpool:
                yr_oi = [dpool.tile([128, 5 * N], F32, name=f"yr_oi{b}",
                                    tag=f"yr_oi{b}") for b in range(BPC)]
                yr_io = [dpool.tile([128, 4 * NO], F32, name=f"yr_io{b}",
                                    tag=f"yr_io{b}") for b in range(BPC)]
                usum = [dpool.tile([128, 5], F32, name=f"usum{b}",
                                   tag=f"usum{b}") for b in range(BPC)]
                gmat = [dpool.tile([128, 5 * 32], F32, name=f"gmat{b}",
                                   tag=f"gmat{b}") for b in range(BPC)]
                exp_sb = [dpool.tile([128, N], F32, name=f"exp{g}", tag=f"exp{g}")
                          for g in range(2)]
                cb_sb = [dpool.tile([128, N], F32, name=f"cb{g}", tag=f"cb{g}")
                         for g in range(2)]
                negmax = [dpool.tile([128, 1], F32, name=f"nm{g}", tag=f"nm{g}")
                          for g in range(2)]
                zsum = [dpool.tile([128, 1], F32, name=f"z{g}", tag=f"z{g}")
                        for g in range(2)]
                rz = [dpool.tile([128, 1], F32, name=f"rz{g}", tag=f"rz{g}")
                      for g in range(2)]
                ebt = [dpool.tile([128, 128], F32, name=f"ebt{g}_{q}",
                                  tag=f"ebt{g}_{q}")
                       for g in range(2) for q in range(4)]
                sm_s = [dpool.tile([128, NO], F32, name=f"sm{g}", tag=f"sm{g}")
                        for g in range(2)]
                sq_s = [dpool.tile([128, NO], F32, name=f"sq{g}", tag=f"sq{g}")
                        for g in range(2)]
                n2 = dpool.tile([128, 2], F32, name="n2", tag="n2")
                t_c = dpool.tile([2, 128], F32, name="t_c", tag="t_c")
                t_d = dpool.tile([2, 128], F32, name="t_d", tag="t_d")
                outt = dpool.tile([2, 128], F32, name="outt", tag="outt")

                ev = _EvictPicker(nc)

                # ======== conv, both orientations, both bgroups ========
                with tc.tile_pool(name="ps_oi", bufs=4, space="PSUM") as poi, \
                     tc.tile_pool(name="ps_io", bufs=2, space="PSUM") as pio:
                    for g in range(2):
                        for c in range(5):
                            for j in range(4):
                                b = 4 * g + j
                                ps = poi.tile([128, N], F32, name="ps_oi",
                                              tag="ps_oi")
                                nc.tensor.matmul(
                                    ps[:],
                                    w_s[32 * j:32 * j + DT + 1,
                                        c * 128:(c + 1) * 128],
                                    xT[g][32 * j:32 * j + DT + 1, :],
                                    start=True, stop=True,
                                    tile_position=(32 * j, 0),
                                )
                                eng = ev.pick(N, need_accum=True)
                                _relu_evict(
                                    nc, eng,
                                    yr_oi[b][:, c * N:(c + 1) * N], ps[:],
                                    accum_out=usum[b][:, c:c + 1],
                                )
                        # gmat[b] = gmask * usum[b] (broadcast over k), bf16
                        for j in range(4):
                            b = 4 * g + j
                            nc.gpsimd.tensor_tensor(
                                out=gmat[b][:].rearrange(
                                    "p (c k) -> p c k", c=5),
                                in0=gmask[:].rearrange(
                                    "p (c k) -> p c k", c=5),
                                in1=usum[b][:].unsqueeze(2).broadcast_to(
                                    [128, 5, 32]),
                                op=OP.mult,
                            )
                        for q in range(4):
                            for j in range(4):
                                b = 4 * g + j
                                ps = pio.tile([128, NO], F32, name="ps_io",
                                              tag="ps_io")
                                for (o0, o1) in ((0, 512), (512, NO)):
                                    nc.tensor.matmul(
                                        ps[:, o0:o1],
                                        xT[g][32 * j:32 * j + DT + 1,
                                              q * 128:(q + 1) * 128],
                                        w_s[32 * j:32 * j + DT + 1, o0:o1],
                                        start=True, stop=True,
                                        tile_position=(32 * j, 0),
                                    )
                                eng = ev.pick(NO)
                                _relu_evict(
                                    nc, eng,
                                    yr_io[b][:, q * NO:(q + 1) * NO], ps[:],
                                )

                # ======== routing: logits+softmax, Cb transpose, S ========
                with tc.tile_pool(name="ps_l", bufs=2, space="PSUM") as pl, \
                     tc.tile_pool(name="ps_t", bufs=2, space="PSUM") as pt, \
                     tc.tile_pool(name="ps_s", bufs=2, space="PSUM") as psp:
                    ps_lg = [pl.tile([128, N], F32, name="ps_l", tag="ps_l")
                             for g in range(2)]
                    for g in range(2):
                        for c in range(5):
                            for j in range(4):
                                b = 4 * g + j
                                nc.tensor.matmul(
                                    ps_lg[g][32 * j:32 * (j + 1), :],
                                    gmat[b][:, c * 32:(c + 1) * 32],
                                    yr_oi[b][:, c * N:(c + 1) * N],
                                    start=(c == 0), stop=(c == 4),
                                    tile_position=(0, 32 * j),
                                    skip_group_check=True,
                                )
                        nc.vector.tensor_reduce(
                            out=negmax[g][:], in_=ps_lg[g][:],
                            op=OP.max, axis=mybir.AxisListType.X, negate=True,
                        )
                        nc.scalar.activation(
                            out=exp_sb[g][:], in_=ps_lg[g][:], func=AF.Exp,
                            bias=negmax[g][:], scale=1.0,
                            accum_out=zsum[g][:],
                        )
                        nc.vector.reciprocal(rz[g][:], zsum[g][:])
                        # Cb = exp/Z + B_bias (garbage rows stay garbage)
                        nc.vector.scalar_tensor_tensor(
                            out=cb_sb[g][:], in0=exp_sb[g][:],
                            scalar=rz[g][:], in1=bb_s[:],
                            op0=OP.mult, op1=OP.add,
                        )
                    # Cb transpose -> ebt tiles [i-part, (j k)] bf16
                    for g in range(2):
                        for q in range(4):
                            tr = pt.tile([128, 128], F32, name="ps_tr",
                                         tag="ps_tr")
                            nc.tensor.transpose(
                                tr[:], cb_sb[g][:, q * 128:(q + 1) * 128],
                                ident[:],
                            )
                            if q % 2 == 0:
                                nc.vector.tensor_copy(ebt[4 * g + q][:], tr[:])
                            else:
                                nc.scalar.copy(ebt[4 * g + q][:], tr[:])
                    # S = sum_q CbT_q^T @ yr_io_q (col-tiled over j)
                    ps_s = [psp.tile([128, NO], F32, name="ps_s", tag="ps_s")
                            for g in range(2)]
                    for g in range(2):
                        for q in range(4):
                            for j in range(4):
                                b = 4 * g + j
                                for (o0, o1) in ((0, 512), (512, NO)):
                                    nc.tensor.matmul(
                                        ps_s[g][32 * j:32 * (j + 1), o0:o1],
                                        ebt[4 * g + q][:, 32 * j:32 * (j + 1)],
                                        yr_io[b][:, q * NO + o0:q * NO + o1],
                                        start=(q == 0), stop=(q == 3),
                                        tile_position=(0, 32 * j),
                                        skip_group_check=True,
                                    )
                    # ======== masked norm + squash tail ========
                    for g in range(2):
                        nc.vector.tensor_tensor(
                            out=sm_s[g][:], in0=ps_s[g][:], in1=smask[:],
                            op=OP.mult,
                        )
                        nc.vector.scalar_tensor_tensor(
                            out=sq_s[g][:], in0=sm_s[g][:],
                            scalar=1.0, in1=sm_s[g][:],
                            op0=OP.mult, op1=OP.mult,
                            accum_out=n2[:, g:g + 1],
                        )
                # transpose n2 -> [2(g), 128(32j+k)] so the output DMA
                # src is a plain partition-range + free-stride AP
                with tc.tile_pool(name="ps_t2", bufs=1, space="PSUM") as pt2:
                    tr2 = pt2.tile([2, 128], F32, name="ps_tr2", tag="ps_tr2")
                    nc.tensor.transpose(tr2[:], n2[:], ident[:])
                    # out = n2/(n2+1) = 1 - 1/(n2+1)
                    nc.vector.tensor_scalar(
                        out=t_c[:], in0=tr2[:], scalar1=1.0, scalar2=None,
                        op0=OP.add)
                    nc.vector.reciprocal(t_d[:], t_c[:])
                    nc.vector.tensor_scalar(
                        out=outt[:], in0=t_d[:], scalar1=-1.0, scalar2=1.0,
                        op0=OP.mult, op1=OP.add)
                    # out[(g j), k] <- outt[g, 32j+k]
                    nc.sync.dma_start(
                        out_d.rearrange("(g j) k -> g j k", g=2),
                        outt[:].rearrange("g (j k) -> g j k", j=4)[:, :, 0:K],
                    )
    nc.compile()
    return nc


_PROGRAM_CACHE = None


def _get_program():
    global _PROGRAM_CACHE
    if _PROGRAM_CACHE is None:
        _PROGRAM_CACHE = _build_program()
    return _PROGRAM_CACHE


def make_in_maps(timecaps, conv_w, conv_b, B_bias):
    """Host-side prep: per-core transposed/padded operand arrays."""
    timecaps = np.ascontiguousarray(np.asarray(timecaps, dtype=np.float32))
    conv_w = np.asarray(conv_w, dtype=np.float32)
    conv_b = np.asarray(conv_b, dtype=np.float32)
    B_bias = np.asarray(B_bias, dtype=np.float32)

    w4 = np.zeros((128, NO), np.float32)
    bb = np.zeros((128, N), np.float32)
    gm = np.zeros((128, 5 * 32), np.float32)
    sm = np.zeros((128, NO), np.float32)
    for j in range(4):
        w4[32 * j:32 * j + DT] = conv_w[:, 0, :]
        w4[32 * j + DT] = conv_b
        bb[32 * j:32 * j + K] = B_bias[:, 0, :]
        sm_rows = sm[32 * j:32 * j + K]
        for k in range(K):
            sm_rows[k, D * k:D * (k + 1)] = 1.0
    for c in range(5):
        for p in range(128):
            gm[p, c * 32 + (c * 128 + p) // D] = 0.25
    ident = np.eye(128, dtype=np.float32)
    w4 = w4.astype(NP_BF16)

    # xT per core: [2, 128, 512], rows 32j+t = x[4g+j, :, t], row 32j+16 = 1
    xs = timecaps.reshape(NCORES, 2, 4, N, DT).transpose(0, 1, 2, 4, 3)
    in_maps = []
    for core in range(NCORES):
        xt = np.zeros((2, 128, N), np.float32)
        for j in range(4):
            xt[:, 32 * j:32 * j + DT] = xs[core, :, j]
            xt[:, 32 * j + DT] = 1.0
        in_maps.append({
            "xt": xt.astype(NP_BF16),
            "w": w4,
            "bb": bb,
            "id": ident,
            "gm": gm,
            "sm": sm,
        })
    return in_maps


def kernel(timecaps, conv_w, conv_b, B_bias):
    nc = _get_program()
    in_maps = make_in_maps(timecaps, conv_w, conv_b, B_bias)
    res = run_bass_kernel_spmd(nc, in_maps, list(range(NCORES)))
    out = np.concatenate([res.results[i]["out"] for i in range(NCORES)], axis=0)
    return out.reshape(B_FULL, K, 1).astype(np.float32)


if __name__ == "__main__":
    rng = np.random.default_rng(0)
    ins = {
        "timecaps": rng.standard_normal((B_FULL, N, DT), dtype=np.float32),
        "conv_w": (rng.standard_normal((DT, 1, NO), dtype=np.float32) * 0.05),
        "conv_b": np.zeros((NO,), dtype=np.float32),
        "B_bias": (rng.standard_normal((K, 1, N), dtype=np.float32) * 0.05),
    }
    print(kernel(**ins)[:2, :, 0])



# revision 4
# speedup vs baseline: 1.0692x; 1.0692x over previous
"""Trainium2 Bass kernel for nn_Classifier (capsule-style conv + routing).

Math (validated against the jax reference, CPU-emulated to 3.4e-3 rel):
  W = conv_w[:,0,:]                                   # [16, 640]
  U[b,m,o]   = relu(sum_t x[b,m,t] W[t,o] + conv_b[o])          (conv as matmul, K=16)
  usum[b,o]  = sum_m U[b,m,o]
  logits     = (usum . U)/4 per class block -> stable softmax over m -> C
  Cb         = C + B_bias[k,m]
  S[b,k,:]   = sum_m Cb[b,k,m] U[b,m,k*64:+64]
  out[b,k]   = n2/(n2+1) with n2 = |S|^2

v3 design (vs v2 at ~85us):
  - ALL matmul operands fp16 (1 pass/row on PE; v2's fp32 routing matmuls
    ran 4 cycles/row).  fp16 (10 mantissa bits) instead of bf16 keeps the
    logit error ~4x smaller; CPU-emulated end-to-end rel err 3.4e-3.
  - U computed ONCE (oi orientation: [o-part, m-free]); relu+eviction
    fused with usum accum_out.  The io orientation ([m-part, o-free],
    needed by the S matmul) is produced by DMA xbar transposes
    (SBUF->SBUF, idle engine) instead of a second conv + a second
    PSUM->SBUF eviction pass (v2 paid ~25us of engine-sum for that).
  - oi layout (c,m), io layout (c,q,o) make every per-(b,c) transpose a
    contiguous [128,512]->[128,512] dma_start_transpose call; the S
    matmul reads io chunks through a 3D access pattern.
  - evictions alternate DVE/ACT (roughly 741ns vs 896ns per tile).
  - Cb transposes output fp16 PSUM (legal for transpose-mode matmuls)
    so their eviction runs in the DVE 2x packed mode.
"""

import numpy as np

import concourse.bass as bass
import concourse.mybir as mybir
import concourse.tile as tile
from concourse import bacc
from concourse.bass_utils import run_bass_kernel_spmd

F32 = mybir.dt.float32
F16 = mybir.dt.float16

B_FULL = 64
N = 512          # num timecaps (routing dim m)
DT = 16          # dim timecaps (conv contraction)
K = 10           # classes
D = 64           # dim classes
NO = K * D       # 640 conv output channels
NCORES = 8
BPC = B_FULL // NCORES   # 8 batches per core
N_WARM = 8               # fp16 N=512 warmup matmuls (~3.4us cold -> HAM warm)

NP_F16 = mybir.dt.np(F16)

AF = mybir.ActivationFunctionType
OP = mybir.AluOpType


def _build_program():
    nc = bacc.Bacc("TRN2", target_bir_lowering=False)
    xt_in = nc.declare_dram_parameter("xt", [2, 128, N], F16, isOutput=False)
    w_in = nc.declare_dram_parameter("w", [128, NO], F16, isOutput=False)
    bb_in = nc.declare_dram_parameter("bb", [128, N], F16, isOutput=False)
    id16_in = nc.declare_dram_parameter("id16", [128, 128], F16, isOutput=False)
    id32_in = nc.declare_dram_parameter("id32", [128, 128], F32, isOutput=False)
    gm_in = nc.declare_dram_parameter("gm", [128, 5 * 32], F16, isOutput=False)
    sm_in = nc.declare_dram_parameter("sm", [128, NO], F16, isOutput=False)
    out_d = nc.declare_dram_parameter("out", [BPC, K], F32, isOutput=True)

    with tile.TileContext(nc) as tc:
        with tc.tile_pool(name="const", bufs=1) as cpool:
            w_s = cpool.tile([128, NO], F16, name="w_s", tag="w_s")
            xT = [cpool.tile([128, N], F16, name=f"xT{g}", tag=f"xT{g}")
                  for g in range(2)]
            gmask = cpool.tile([128, 5 * 32], F16, name="gmask", tag="gmask")
            bb_s = cpool.tile([128, N], F16, name="bb_s", tag="bb_s")
            ident16 = cpool.tile([128, 128], F16, name="ident16", tag="ident16")
            ident32 = cpool.tile([128, 128], F32, name="ident32", tag="ident32")
            smask = cpool.tile([128, NO], F16, name="smask", tag="smask")

            # DMA order = priority order: warmup operand first, then conv
            # operands, then late-phase constants.
            nc.sync.dma_start(w_s[:], w_in[:, :])
            for g in range(2):
                nc.sync.dma_start(xT[g][:], xt_in[g])
            nc.sync.dma_start(gmask[:], gm_in[:, :])
            nc.sync.dma_start(bb_s[:], bb_in[:, :])
            nc.sync.dma_start(ident16[:], id16_in[:, :])
            nc.sync.dma_start(ident32[:], id32_in[:, :])
            nc.sync.dma_start(smask[:], sm_in[:, :])

            # ---- HAM warmup: fp16 N=512 matmuls on the first-arrived w ----
            with tc.tile_pool(name="ps_warm", bufs=1, space="PSUM") as pw:
                ps_w = pw.tile([128, N], F32, name="ps_w", tag="ps_w")
                for r in range(N_WARM):
                    nc.tensor.matmul(
                        ps_w[:], w_s[0:128, 0:128], w_s[0:128, 0:N],
                        start=(r == 0), stop=(r == N_WARM - 1),
                    )

            with tc.tile_pool(name="data", bufs=1) as dpool:
                yr_oi = [dpool.tile([128, 5 * N], F16, name=f"yr_oi{b}",
                                    tag=f"yr_oi{b}") for b in range(BPC)]
                yr_io = [dpool.tile([128, 5 * N], F16, name=f"yr_io{b}",
                                    tag=f"yr_io{b}") for b in range(BPC)]
                usum = dpool.tile([128, 5 * BPC], F32, name="usum", tag="usum")
                usum16 = [dpool.tile([128, 5], F16, name=f"usum16_{b}",
                                     tag=f"usum16_{b}") for b in range(BPC)]
                gmat = [dpool.tile([128, 5 * 32], F16, name=f"gmat{b}",
                                   tag=f"gmat{b}") for b in range(BPC)]
                exp_sb = [dpool.tile([128, N], F16, name=f"exp{g}", tag=f"exp{g}")
                          for g in range(2)]
                negmax = [dpool.tile([128, 1], F32, name=f"nm{g}", tag=f"nm{g}")
                          for g in range(2)]
                zsum = [dpool.tile([128, 1], F32, name=f"z{g}", tag=f"z{g}")
                        for g in range(2)]
                rz = [dpool.tile([128, 1], F32, name=f"rz{g}", tag=f"rz{g}")
                      for g in range(2)]
                cb_sb = [dpool.tile([128, N], F16, name=f"cb{g}", tag=f"cb{g}")
                         for g in range(2)]
                ebt_sb = [dpool.tile([128, N], F16, name=f"ebt{g}", tag=f"ebt{g}")
                          for g in range(2)]
                s_sb = [dpool.tile([128, NO], F16, name=f"s{g}", tag=f"s{g}")
                        for g in range(2)]
                n2 = dpool.tile([128, 2], F32, name="n2", tag="n2")
                t_c = dpool.tile([2, 128], F32, name="t_c", tag="t_c")
                t_d = dpool.tile([2, 128], F32, name="t_d", tag="t_d")
                outt = dpool.tile([2, 128], F32, name="outt", tag="outt")

                evict_idx = 0

                with tc.tile_pool(name="ps_conv", bufs=4, space="PSUM") as pcv, \
                     tc.tile_pool(name="ps_lg", bufs=1, space="PSUM") as plg, \
                     tc.tile_pool(name="ps_ebt", bufs=1, space="PSUM") as peb, \
                     tc.tile_pool(name="ps_s", bufs=1, space="PSUM") as psp:
                    for g in range(2):
                        # ======== conv (oi) + fused relu/usum eviction ======
                        for c in range(5):
                            tiles = []
                            for j in range(4):
                                ps = pcv.tile([128, N], F32, name="ps_cv",
                                              tag="ps_cv")
                                nc.tensor.matmul(
                                    ps[:],
                                    w_s[32 * j:32 * j + DT + 1,
                                        c * 128:(c + 1) * 128],
                                    xT[g][32 * j:32 * j + DT + 1, :],
                                    start=True, stop=True,
                                    tile_position=(32 * j, 0),
                                )
                                tiles.append(ps)
                            for j in range(4):
                                b = 4 * g + j
                                dst = yr_oi[b][:, c * N:(c + 1) * N]
                                acc = usum[:, b * 5 + c:b * 5 + c + 1]
                                if evict_idx % 2 == 0:
                                    nc.vector.tensor_scalar(
                                        out=dst, in0=tiles[j][:],
                                        scalar1=0.0, scalar2=0.0,
                                        op0=OP.max, op1=OP.add,
                                        accum_out=acc,
                                    )
                                else:
                                    nc.scalar.activation(
                                        out=dst, in_=tiles[j][:],
                                        func=AF.Relu, accum_out=acc,
                                    )
                                evict_idx += 1
                                # io orientation via DMA xbar transpose.
                                # 3D out AP => full [512,128] transpose with
                                # out[p, q, oc] = in[oc, q*128+p]  (m=q*128+p)
                                nc.sync.dma_start_transpose(
                                    yr_io[b][:, c * N:(c + 1) * N].rearrange(
                                        "p (q o) -> p q o", q=4),
                                    dst)

                        # ======== gmat = gmask * usum (per batch) ========
                        for j in range(4):
                            b = 4 * g + j
                            nc.vector.tensor_copy(
                                usum16[b][:], usum[:, b * 5:b * 5 + 5])
                            nc.gpsimd.tensor_tensor(
                                out=gmat[b][:].rearrange(
                                    "p (c k) -> p c k", c=5),
                                in0=gmask[:].rearrange(
                                    "p (c k) -> p c k", c=5),
                                in1=usum16[b][:].unsqueeze(2).broadcast_to(
                                    [128, 5, 32]),
                                op=OP.mult,
                            )

                        # ======== logits (col-tiled over j) ========
                        ps_lg = plg.tile([128, N], F32, name="ps_lg",
                                         tag="ps_lg")
                        for c in range(5):
                            for j in range(4):
                                b = 4 * g + j
                                nc.tensor.matmul(
                                    ps_lg[32 * j:32 * (j + 1), :],
                                    gmat[b][:, c * 32:(c + 1) * 32],
                                    yr_oi[b][:, c * N:(c + 1) * N],
                                    start=(c == 0), stop=(c == 4),
                                    tile_position=(0, 32 * j),
                                    skip_group_check=True,
                                )

                        # ======== softmax -> Cb ========
                        nc.vector.tensor_reduce(
                            out=negmax[g][:], in_=ps_lg[:],
                            op=OP.max, axis=mybir.AxisListType.X, negate=True,
                        )
                        nc.scalar.activation(
                            out=exp_sb[g][:], in_=ps_lg[:], func=AF.Exp,
                            bias=negmax[g][:], scale=1.0,
                            accum_out=zsum[g][:],
                        )
                        nc.vector.reciprocal(rz[g][:], zsum[g][:])
                        nc.vector.scalar_tensor_tensor(
                            out=cb_sb[g][:], in0=exp_sb[g][:],
                            scalar=rz[g][:], in1=bb_s[:],
                            op0=OP.mult, op1=OP.add,
                        )

                        # ======== Cb transpose (fp16 PSUM) ========
                        ps_eb = peb.tile([128, N], F16, name="ps_eb",
                                         tag="ps_eb")
                        for q in range(4):
                            nc.tensor.transpose(
                                ps_eb[:, q * 128:(q + 1) * 128],
                                cb_sb[g][:, q * 128:(q + 1) * 128],
                                ident16[:],
                            )
                        nc.vector.tensor_copy(ebt_sb[g][:], ps_eb[:])

                        # ======== S = Cb @ U (col-tiled over j) ========
                        ps_s = psp.tile([128, NO], F32, name="ps_s", tag="ps_s")
                        for q in range(4):
                            for j in range(4):
                                b = 4 * g + j
                                io4 = yr_io[b][:].rearrange(
                                    "p (c q o) -> p c q o", c=5, q=4)
                                nc.tensor.matmul(
                                    ps_s[32 * j:32 * (j + 1), 0:512],
                                    ebt_sb[g][:, q * 128 + 32 * j:
                                              q * 128 + 32 * (j + 1)],
                                    io4[:, 0:4, q, :],
                                    start=(q == 0), stop=(q == 3),
                                    tile_position=(0, 32 * j),
                                    skip_group_check=True,
                                )
                                nc.tensor.matmul(
                                    ps_s[32 * j:32 * (j + 1), 512:NO],
                                    ebt_sb[g][:, q * 128 + 32 * j:
                                              q * 128 + 32 * (j + 1)],
                                    io4[:, 4, q, :],
                                    start=(q == 0), stop=(q == 3),
                                    tile_position=(0, 32 * j),
                                    skip_group_check=True,
                                )

                        # ======== masked squash tail ========
                        nc.vector.tensor_tensor(
                            out=s_sb[g][:], in0=ps_s[:], in1=smask[:],
                            op=OP.mult,
                        )
                        nc.vector.scalar_tensor_tensor(
                            out=s_sb[g][:], in0=s_sb[g][:],
                            scalar=1.0, in1=s_sb[g][:],
                            op0=OP.mult, op1=OP.mult,
                            accum_out=n2[:, g:g + 1],
                        )

                # transpose n2 -> [2(g), 128(32j+k)]; out = 1 - 1/(n2+1)
                with tc.tile_pool(name="ps_t2", bufs=1, space="PSUM") as pt2:
                    tr2 = pt2.tile([2, 128], F32, name="ps_tr2", tag="ps_tr2")
                    nc.tensor.transpose(tr2[:], n2[:], ident32[:])
                    nc.vector.tensor_scalar(
                        out=t_c[:], in0=tr2[:], scalar1=1.0, scalar2=None,
                        op0=OP.add)
                    nc.vector.reciprocal(t_d[:], t_c[:])
                    nc.vector.tensor_scalar(
                        out=outt[:], in0=t_d[:], scalar1=-1.0, scalar2=1.0,
                        op0=OP.mult, op1=OP.add)
                    nc.sync.dma_start(
                        out_d.rearrange("(g j) k -> g j k", g=2),
                        outt[:].rearrange("g (j k) -> g j k", j=4)[:, :, 0:K],
                    )
    nc.compile()
    return nc


_PROGRAM_CACHE = None


def _get_program():
    global _PROGRAM_CACHE
    if _PROGRAM_CACHE is None:
        _PROGRAM_CACHE = _build_program()
    return _PROGRAM_CACHE


def make_in_maps(timecaps, conv_w, conv_b, B_bias):
    """Host-side prep: per-core transposed/padded operand arrays."""
    timecaps = np.ascontiguousarray(np.asarray(timecaps, dtype=np.float32))
    conv_w = np.asarray(conv_w, dtype=np.float32)
    conv_b = np.asarray(conv_b, dtype=np.float32)
    B_bias = np.asarray(B_bias, dtype=np.float32)

    w4 = np.zeros((128, NO), np.float32)
    bb = np.zeros((128, N), np.float32)
    gm = np.zeros((128, 5 * 32), np.float32)
    sm = np.zeros((128, NO), np.float32)
    for j in range(4):
        w4[32 * j:32 * j + DT] = conv_w[:, 0, :]
        w4[32 * j + DT] = conv_b
        bb[32 * j:32 * j + K] = B_bias[:, 0, :]
        sm_rows = sm[32 * j:32 * j + K]
        for k in range(K):
            sm_rows[k, D * k:D * (k + 1)] = 1.0
    for c in range(5):
        for p in range(128):
            gm[p, c * 32 + (c * 128 + p) // D] = 0.25
    ident = np.eye(128, dtype=np.float32)

    # xT per core: [2, 128, 512], rows 32j+t = x[4g+j, :, t], row 32j+16 = 1
    xs = timecaps.reshape(NCORES, 2, 4, N, DT).transpose(0, 1, 2, 4, 3)
    in_maps = []
    shared = {
        "w": w4.astype(NP_F16),
        "bb": bb.astype(NP_F16),
        "id16": ident.astype(NP_F16),
        "id32": ident,
        "gm": gm.astype(NP_F16),
        "sm": sm.astype(NP_F16),
    }
    for core in range(NCORES):
        xt = np.zeros((2, 128, N), np.float32)
        for j in range(4):
            xt[:, 32 * j:32 * j + DT] = xs[core, :, j]
            xt[:, 32 * j + DT] = 1.0
        in_maps.append({"xt": xt.astype(NP_F16), **shared})
    return in_maps


def kernel(timecaps, conv_w, conv_b, B_bias):
    nc = _get_program()
    in_maps = make_in_maps(timecaps, conv_w, conv_b, B_bias)
    res = run_bass_kernel_spmd(nc, in_maps, list(range(NCORES)))
    out = np.concatenate([res.results[i]["out"] for i in range(NCORES)], axis=0)
    return out.reshape(B_FULL, K, 1).astype(np.float32)


if __name__ == "__main__":
    rng = np.random.default_rng(0)
    ins = {
        "timecaps": rng.standard_normal((B_FULL, N, DT), dtype=np.float32),
        "conv_w": (rng.standard_normal((DT, 1, NO), dtype=np.float32) * 0.05),
        "conv_b": np.zeros((NO,), dtype=np.float32),
        "B_bias": (rng.standard_normal((K, 1, N), dtype=np.float32) * 0.05),
    }
    print(kernel(**ins)[:2, :, 0])


# revision 16
# speedup vs baseline: 1.2251x; 1.1458x over previous
"""Trainium2 Bass kernel for nn_Classifier (capsule-style conv + routing).

Math (validated against the jax reference, CPU-emulated to 3.4e-3 rel):
  W = conv_w[:,0,:]                                   # [16, 640]
  U[b,m,o]   = relu(sum_t x[b,m,t] W[t,o] + conv_b[o])          (conv as matmul, K=16)
  usum[b,o]  = sum_m U[b,m,o]
  logits     = (usum . U)/4 per class block -> stable softmax over m -> C
  Cb         = C + B_bias[k,m]
  S[b,k,:]   = sum_m Cb[b,k,m] U[b,m,k*64:+64]
  out[b,k]   = n2/(n2+1) with n2 = |S|^2

v3 design (vs v2 at ~85us):
  - ALL matmul operands fp16 (1 pass/row on PE; v2's fp32 routing matmuls
    ran 4 cycles/row).  fp16 (10 mantissa bits) instead of bf16 keeps the
    logit error ~4x smaller; CPU-emulated end-to-end rel err 3.4e-3.
  - U computed ONCE (oi orientation: [o-part, m-free]); relu+eviction
    fused with usum accum_out.  The io orientation ([m-part, o-free],
    needed by the S matmul) is produced by DMA xbar transposes
    (SBUF->SBUF, idle engine) instead of a second conv + a second
    PSUM->SBUF eviction pass (v2 paid ~25us of engine-sum for that).
  - oi layout (c,m), io layout (c,q,o) make every per-(b,c) transpose a
    contiguous [128,512]->[128,512] dma_start_transpose call; the S
    matmul reads io chunks through a 3D access pattern.
  - evictions alternate DVE/ACT (roughly 741ns vs 896ns per tile).
  - Cb transposes output fp16 PSUM (legal for transpose-mode matmuls)
    so their eviction runs in the DVE 2x packed mode.
"""

import numpy as np

import concourse.bass as bass
import concourse.mybir as mybir
import concourse.tile as tile
from concourse import bacc
from concourse.bass_utils import run_bass_kernel_spmd

F32 = mybir.dt.float32
F16 = mybir.dt.float16

B_FULL = 64
N = 512          # num timecaps (routing dim m)
DT = 16          # dim timecaps (conv contraction)
K = 10           # classes
D = 64           # dim classes
NO = K * D       # 640 conv output channels
NCORES = 8
BPC = B_FULL // NCORES   # 8 batches per core
N_WARM = 8               # fp16 N=512 warmup matmuls (~3.4us cold -> HAM warm)

NP_F16 = mybir.dt.np(F16)

AF = mybir.ActivationFunctionType
OP = mybir.AluOpType


def _build_program():
    nc = bacc.Bacc("TRN2", target_bir_lowering=False)
    xt_in = nc.declare_dram_parameter("xt", [2, 128, N], F16, isOutput=False)
    w_in = nc.declare_dram_parameter("w", [128, NO], F16, isOutput=False)
    bb_in = nc.declare_dram_parameter("bb", [128, N], F16, isOutput=False)
    id16_in = nc.declare_dram_parameter("id16", [128, 128], F16, isOutput=False)
    id32_in = nc.declare_dram_parameter("id32", [128, 128], F32, isOutput=False)
    gm_in = nc.declare_dram_parameter("gm", [128, 5 * 32], F16, isOutput=False)
    sm_in = nc.declare_dram_parameter("sm", [128, NO], F16, isOutput=False)
    out_d = nc.declare_dram_parameter("out", [BPC, K], F32, isOutput=True)

    with tile.TileContext(nc) as tc:
        with tc.tile_pool(name="const", bufs=1) as cpool:
            w_s = cpool.tile([128, NO], F16, name="w_s", tag="w_s")
            xT = [cpool.tile([128, N], F16, name=f"xT{g}", tag=f"xT{g}")
                  for g in range(2)]
            gmask = cpool.tile([128, 5 * 32], F16, name="gmask", tag="gmask")
            bb_s = cpool.tile([128, N], F16, name="bb_s", tag="bb_s")
            ident16 = cpool.tile([128, 128], F16, name="ident16", tag="ident16")
            ident32 = cpool.tile([128, 128], F32, name="ident32", tag="ident32")
            smask = cpool.tile([128, NO], F16, name="smask", tag="smask")

            # DMA order = priority order: warmup operand first, then conv
            # operands, then late-phase constants.
            nc.sync.dma_start(w_s[:], w_in[:, :])
            for g in range(2):
                nc.sync.dma_start(xT[g][:], xt_in[g])
            nc.sync.dma_start(gmask[:], gm_in[:, :])
            nc.sync.dma_start(bb_s[:], bb_in[:, :])
            nc.sync.dma_start(ident16[:], id16_in[:, :])
            nc.sync.dma_start(ident32[:], id32_in[:, :])
            nc.sync.dma_start(smask[:], sm_in[:, :])

            # ---- HAM warmup: fp16 N=512 matmuls on the first-arrived w ----
            with tc.tile_pool(name="ps_warm", bufs=1, space="PSUM") as pw:
                ps_w = pw.tile([128, N], F32, name="ps_w", tag="ps_w")
                for r in range(N_WARM):
                    nc.tensor.matmul(
                        ps_w[:], w_s[0:128, 0:128], w_s[0:128, 0:N],
                        start=(r == 0), stop=(r == N_WARM - 1),
                    )

            with tc.tile_pool(name="data", bufs=1) as dpool:
                yr_oi = [dpool.tile([128, 5 * N], F16, name=f"yr_oi{b}",
                                    tag=f"yr_oi{b}") for b in range(BPC)]
                # io layout: one tile [128, (b, q, 640)] so multi-batch
                # eviction dests are single strided APs
                yr_io_all = dpool.tile([128, BPC * 4 * NO], F16,
                                       name="yr_io_all", tag="yr_io_all")
                yr_io = [yr_io_all[:, b * 4 * NO:(b + 1) * 4 * NO]
                         for b in range(BPC)]
                usum = dpool.tile([128, 5 * BPC], F32, name="usum", tag="usum")
                usum16 = [dpool.tile([128, 5], F16, name=f"usum16_{b}",
                                     tag=f"usum16_{b}") for b in range(BPC)]
                gmat = [dpool.tile([128, 5 * 32], F16, name=f"gmat{b}",
                                   tag=f"gmat{b}") for b in range(BPC)]
                exp_sb = [dpool.tile([128, N], F16, name=f"exp{g}", tag=f"exp{g}")
                          for g in range(2)]
                negmax = [dpool.tile([128, 1], F32, name=f"nm{g}", tag=f"nm{g}")
                          for g in range(2)]
                zsum = [dpool.tile([128, 1], F32, name=f"z{g}", tag=f"z{g}")
                        for g in range(2)]
                rz = [dpool.tile([128, 1], F32, name=f"rz{g}", tag=f"rz{g}")
                      for g in range(2)]
                cb_sb = [dpool.tile([128, N], F16, name=f"cb{g}", tag=f"cb{g}")
                         for g in range(2)]
                ebt_sb = [dpool.tile([128, N], F16, name=f"ebt{g}", tag=f"ebt{g}")
                          for g in range(2)]
                s_sb = [dpool.tile([128, NO], F16, name=f"s{g}", tag=f"s{g}")
                        for g in range(2)]
                n2 = dpool.tile([128, 2], F32, name="n2", tag="n2")
                t_c = dpool.tile([2, 128], F32, name="t_c", tag="t_c")
                t_d = dpool.tile([2, 128], F32, name="t_d", tag="t_d")
                outt = dpool.tile([2, 128], F32, name="outt", tag="outt")

                evict_idx = 0

                def evict(dst, src, acc=None):
                    nonlocal evict_idx
                    if evict_idx % 2 == 0:
                        if acc is not None:
                            nc.vector.tensor_scalar(
                                out=dst, in0=src, scalar1=0.0, scalar2=0.0,
                                op0=OP.max, op1=OP.add, accum_out=acc)
                        else:
                            nc.vector.tensor_scalar(
                                out=dst, in0=src, scalar1=0.0, scalar2=None,
                                op0=OP.max)
                    else:
                        nc.scalar.activation(
                            out=dst, in_=src, func=AF.Relu, accum_out=acc)
                    evict_idx += 1

                # PSUM: cv ring 4 + lg 1 + ebt 1 + s 2 = 8 banks
                with tc.tile_pool(name="ps_conv", bufs=4, space="PSUM") as pcv, \
                     tc.tile_pool(name="ps_lg", bufs=1, space="PSUM") as plg, \
                     tc.tile_pool(name="ps_ebt", bufs=1, space="PSUM") as peb, \
                     tc.tile_pool(name="ps_s", bufs=1, space="PSUM") as psp:
                    # ===== phase 1 per g: conv-oi, usum, logits, softmax =====
                    for g in range(2):
                        for c in range(5):
                            tiles = []
                            for j in range(4):
                                ps = pcv.tile([128, N], F32, name="ps_cv",
                                              tag="ps_cv")
                                nc.tensor.matmul(
                                    ps[:],
                                    w_s[32 * j:32 * j + DT + 1,
                                        c * 128:(c + 1) * 128],
                                    xT[g][32 * j:32 * j + DT + 1, :],
                                    start=True, stop=True,
                                    tile_position=(32 * j, 0),
                                )
                                tiles.append(ps)
                            for j in range(4):
                                b = 4 * g + j
                                evict(yr_oi[b][:, c * N:(c + 1) * N],
                                      tiles[j][:],
                                      usum[:, b * 5 + c:b * 5 + c + 1])

                        # gmat = gmask * usum (per batch, fp16 on gpsimd)
                        for j in range(4):
                            b = 4 * g + j
                            nc.vector.tensor_copy(
                                usum16[b][:], usum[:, b * 5:b * 5 + 5])
                            nc.gpsimd.tensor_tensor(
                                out=gmat[b][:].rearrange(
                                    "p (c k) -> p c k", c=5),
                                in0=gmask[:].rearrange(
                                    "p (c k) -> p c k", c=5),
                                in1=usum16[b][:].unsqueeze(2).broadcast_to(
                                    [128, 5, 32]),
                                op=OP.mult,
                            )

                        # logits (col-tiled over j)
                        ps_lg = plg.tile([128, N], F32, name="ps_lg",
                                         tag="ps_lg")
                        for c in range(5):
                            for j in range(4):
                                b = 4 * g + j
                                nc.tensor.matmul(
                                    ps_lg[32 * j:32 * (j + 1), :],
                                    gmat[b][:, c * 32:(c + 1) * 32],
                                    yr_oi[b][:, c * N:(c + 1) * N],
                                    start=(c == 0), stop=(c == 4),
                                    tile_position=(0, 32 * j),
                                    skip_group_check=True,
                                )

                        # softmax -> Cb
                        nc.vector.tensor_reduce(
                            out=negmax[g][:], in_=ps_lg[:],
                            op=OP.max, axis=mybir.AxisListType.X, negate=True,
                        )
                        nc.scalar.activation(
                            out=exp_sb[g][:], in_=ps_lg[:], func=AF.Exp,
                            bias=negmax[g][:], scale=1.0,
                            accum_out=zsum[g][:],
                        )
                        nc.vector.reciprocal(rz[g][:], zsum[g][:])
                        nc.vector.scalar_tensor_tensor(
                            out=cb_sb[g][:], in0=exp_sb[g][:],
                            scalar=rz[g][:], in1=bb_s[:],
                            op0=OP.mult, op1=OP.add,
                        )

                        # Cb transpose (fp16 PSUM -> 2x eviction)
                        ps_eb = peb.tile([128, N], F16, name="ps_eb",
                                         tag="ps_eb")
                        for q in range(4):
                            nc.tensor.transpose(
                                ps_eb[:, q * 128:(q + 1) * 128],
                                cb_sb[g][:, q * 128:(q + 1) * 128],
                                ident16[:],
                            )
                        nc.vector.tensor_copy(ebt_sb[g][:], ps_eb[:])

                    # ===== phase 2 per g: conv-io, S, squash tail =====
                    # yr_io layout per batch: [128(m of chunk q), (q, 640 o)]
                    for g in range(2):
                        ps_s = psp.tile([128, NO], F32, name="ps_s", tag="ps_s")
                        for q in range(4):
                            tiles = []
                            for j in range(4):
                                ps = pcv.tile([128, N], F32, name="ps_io",
                                              tag="ps_cv")
                                nc.tensor.matmul(
                                    ps[:],
                                    xT[g][32 * j:32 * j + DT + 1,
                                          q * 128:(q + 1) * 128],
                                    w_s[32 * j:32 * j + DT + 1, 0:512],
                                    start=True, stop=True,
                                    tile_position=(32 * j, 0),
                                )
                                tiles.append(ps)
                            for j in range(4):
                                b = 4 * g + j
                                evict(yr_io[b][:, q * NO:q * NO + 512],
                                      tiles[j][:])
                            # o-tail (cols 512:640) via PE transposes of the
                            # already-evicted yr_oi c=4 chunk: sequential
                            # writes into one fp16 PSUM bank, 2x eviction
                            ps_tl = peb.tile([128, N], F16, name="ps_tl",
                                             tag="ps_eb")
                            for j in range(4):
                                b = 4 * g + j
                                nc.tensor.transpose(
                                    ps_tl[:, j * 128:(j + 1) * 128],
                                    yr_oi[b][:, 4 * N + q * 128:
                                             4 * N + (q + 1) * 128],
                                    ident16[:],
                                )
                            evict(yr_io_all[:].rearrange(
                                      "p (b q o) -> p b q o", b=BPC, q=4)
                                  [:, 4 * g:4 * g + 4, q, 512:NO],
                                  ps_tl[:])
                            # S matmuls for this q (col-tiled over j)
                            for j in range(4):
                                b = 4 * g + j
                                nc.tensor.matmul(
                                    ps_s[32 * j:32 * (j + 1), 0:512],
                                    ebt_sb[g][:, q * 128 + 32 * j:
                                              q * 128 + 32 * (j + 1)],
                                    yr_io[b][:, q * NO:q * NO + 512],
                                    start=(q == 0), stop=(q == 3),
                                    tile_position=(0, 32 * j),
                                    skip_group_check=True,
                                )
                                nc.tensor.matmul(
                                    ps_s[32 * j:32 * (j + 1), 512:NO],
                                    ebt_sb[g][:, q * 128 + 32 * j:
                                              q * 128 + 32 * (j + 1)],
                                    yr_io[b][:, q * NO + 512:q * NO + NO],
                                    start=(q == 0), stop=(q == 3),
                                    tile_position=(0, 32 * j),
                                    skip_group_check=True,
                                )

                        # masked squash tail
                        nc.vector.tensor_tensor(
                            out=s_sb[g][:], in0=ps_s[:], in1=smask[:],
                            op=OP.mult,
                        )
                        nc.vector.scalar_tensor_tensor(
                            out=s_sb[g][:], in0=s_sb[g][:],
                            scalar=1.0, in1=s_sb[g][:],
                            op0=OP.mult, op1=OP.mult,
                            accum_out=n2[:, g:g + 1],
                        )

                # transpose n2 -> [2(g), 128(32j+k)]; out = 1 - 1/(n2+1)
                with tc.tile_pool(name="ps_t2", bufs=1, space="PSUM") as pt2:
                    tr2 = pt2.tile([2, 128], F32, name="ps_tr2", tag="ps_tr2")
                    nc.tensor.transpose(tr2[:], n2[:], ident32[:])
                    nc.vector.tensor_scalar(
                        out=t_c[:], in0=tr2[:], scalar1=1.0, scalar2=None,
                        op0=OP.add)
                    nc.vector.reciprocal(t_d[:], t_c[:])
                    nc.vector.tensor_scalar(
                        out=outt[:], in0=t_d[:], scalar1=-1.0, scalar2=1.0,
                        op0=OP.mult, op1=OP.add)
                    nc.sync.dma_start(
                        out_d.rearrange("(g j) k -> g j k", g=2),
                        outt[:].rearrange("g (j k) -> g j k", j=4)[:, :, 0:K],
                    )
    nc.compile()
    return nc


_PROGRAM_CACHE = None


def _get_program():
    global _PROGRAM_CACHE
    if _PROGRAM_CACHE is None:
        _PROGRAM_CACHE = _build_program()
    return _PROGRAM_CACHE


def make_in_maps(timecaps, conv_w, conv_b, B_bias):
    """Host-side prep: per-core transposed/padded operand arrays."""
    timecaps = np.ascontiguousarray(np.asarray(timecaps, dtype=np.float32))
    conv_w = np.asarray(conv_w, dtype=np.float32)
    conv_b = np.asarray(conv_b, dtype=np.float32)
    B_bias = np.asarray(B_bias, dtype=np.float32)

    w4 = np.zeros((128, NO), np.float32)
    bb = np.zeros((128, N), np.float32)
    gm = np.zeros((128, 5 * 32), np.float32)
    sm = np.zeros((128, NO), np.float32)
    for j in range(4):
        w4[32 * j:32 * j + DT] = conv_w[:, 0, :]
        w4[32 * j + DT] = conv_b
        bb[32 * j:32 * j + K] = B_bias[:, 0, :]
        sm_rows = sm[32 * j:32 * j + K]
        for k in range(K):
            sm_rows[k, D * k:D * (k + 1)] = 1.0
    for c in range(5):
        for p in range(128):
            gm[p, c * 32 + (c * 128 + p) // D] = 0.25
    ident = np.eye(128, dtype=np.float32)

    # xT per core: [2, 128, 512], rows 32j+t = x[4g+j, :, t], row 32j+16 = 1
    xs = timecaps.reshape(NCORES, 2, 4, N, DT).transpose(0, 1, 2, 4, 3)
    in_maps = []
    shared = {
        "w": w4.astype(NP_F16),
        "bb": bb.astype(NP_F16),
        "id16": ident.astype(NP_F16),
        "id32": ident,
        "gm": gm.astype(NP_F16),
        "sm": sm.astype(NP_F16),
    }
    for core in range(NCORES):
        xt = np.zeros((2, 128, N), np.float32)
        for j in range(4):
            xt[:, 32 * j:32 * j + DT] = xs[core, :, j]
            xt[:, 32 * j + DT] = 1.0
        in_maps.append({"xt": xt.astype(NP_F16), **shared})
    return in_maps


def kernel(timecaps, conv_w, conv_b, B_bias):
    nc = _get_program()
    in_maps = make_in_maps(timecaps, conv_w, conv_b, B_bias)
    res = run_bass_kernel_spmd(nc, in_maps, list(range(NCORES)))
    out = np.concatenate([res.results[i]["out"] for i in range(NCORES)], axis=0)
    return out.reshape(B_FULL, K, 1).astype(np.float32)


if __name__ == "__main__":
    rng = np.random.default_rng(0)
    ins = {
        "timecaps": rng.standard_normal((B_FULL, N, DT), dtype=np.float32),
        "conv_w": (rng.standard_normal((DT, 1, NO), dtype=np.float32) * 0.05),
        "conv_b": np.zeros((NO,), dtype=np.float32),
        "B_bias": (rng.standard_normal((K, 1, N), dtype=np.float32) * 0.05),
    }
    print(kernel(**ins)[:2, :, 0])
